# revision 30
# baseline (speedup 1.0000x reference)
"""nn_MultiHeadAttention TRN2 kernel: 8-core tensor-parallel (2 heads/core).

Self-contained: builds and compiles the Bass/Tile SPMD program on first call,
shards the full inputs per-core on the host, runs via run_bass_kernel_spmd,
and concatenates the per-core sequence-block outputs into the full output.

Algorithm (per core, 2 heads of 16, head_dim 64, S=4096, D=1024):
  - feature-major layout: xP [128, 8, S] (partition, feature-tile, seq) so one
    1MB DMA delivers a full 512-query chunk across all 8 feature tiles;
    projections for chunk 0 start ~4us in.  Weights are host-packed so each
    loads in a single descriptor.
  - q/k projected with RoPE-permuted, transposed weight shards; the rotary
    partner permutation makes the rotation a 16-lane half-swap within every
    32-partition block (one DVE stream_shuffle), signs folded into sin.
  - chunked pipeline: per 512-query chunk, project q/k (PE), evacuate+RoPE
    (DVE), project v into an augmented seq-major tile with a ones column
    (softmax denominator); attention for query block Q starts as soon as its
    chunks are ready.
  - flash attention on transposed score tiles scoresT[k,q]: scores for a
    jt-pair land in a [128,1024] PSUM tile (3-slot rotation), causal mask
    added as a -400 triangle on diagonal tiles (DVE), exp on ScalarE over
    [128,1024] per head-pair, PV accumulates outT[128,512] per head (row 64
    = softmax denominator, rows 65-127 padding from the 128-wide stationary).
  - normalize, staged over the next block so it never blocks the PE or DVE:
    outT evacuates to SBUF immediately (PSUM recycles), two 1-row DVE
    reciprocals produce the inverse denominators, one bf16 128x128
    select-matmul broadcasts them across partitions, two DVE multiplies;
    per-block staging DMA feeds the AllToAll input.
  - startup DMAs/memsets spread across the SP/ACT/DVE/GpSimd queues (the
    ~0.6us per-descriptor enqueue is serial per queue).
  - AllToAll re-shards from head-split to sequence-split; final projection
    against full Wo.T; each core emits out[512, 1024] f32.
"""

from contextlib import ExitStack

import numpy as np
import ml_dtypes

import concourse.tile as tile
from concourse import bacc, mybir
from concourse.bass_utils import run_bass_kernel_spmd

F32 = mybir.dt.float32
BF16 = mybir.dt.bfloat16

S = 4096
D = 1024
HD = 64
N_CORES = 8
KT = 128
BQ = 512
NFT = D // 128          # 8 feature tiles
NKT = S // KT           # 32 key tiles
NQB = S // BQ           # 8 query blocks
CHUNK = S // N_CORES    # 512

SHUF_HALF = list(range(16, 32)) + list(range(16))


def _build():
    nc = bacc.Bacc("TRN2", target_bir_lowering=False, debug=False, num_devices=N_CORES)

    xP = nc.dram_tensor("xP", [NQB, 128, NFT, BQ], BF16, kind="ExternalInput")
    wq = nc.dram_tensor("wq", [128, NFT * 128], BF16, kind="ExternalInput")
    wk = nc.dram_tensor("wk", [128, NFT * 128], BF16, kind="ExternalInput")
    wv = nc.dram_tensor("wv", [128, NFT * 128], BF16, kind="ExternalInput")
    wo = nc.dram_tensor("wo", [128, NFT, D], BF16, kind="ExternalInput")
    cosP = nc.dram_tensor("cosP", [128, S], BF16, kind="ExternalInput")
    sinN = nc.dram_tensor("sinN", [128, S], BF16, kind="ExternalInput")
    lu = nc.dram_tensor("lu", [128, 128], BF16, kind="ExternalInput")
    out = nc.dram_tensor("out", [CHUNK, D], F32, kind="ExternalOutput")

    # split AllToAll: #1 covers query blocks 0-3 (fires mid-kernel, hides the
    # inter-core skew + CC setup behind attention of blocks 4-7), #2 covers
    # blocks 4-7 in the tail.  Layout [2, 512, 256]: chunk d = (query-half
    # d//4, block d%4) -> core d owns queries 512*(d%4) + 256*(d//4) + [0,256)
    # of each 2048-query group.
    a2a1_in = nc.dram_tensor("a2a1_in", [2, 4 * 128, 256], BF16)
    a2a1_out = nc.dram_tensor("a2a1_out", [2, 4 * 128, 256], BF16)
    a2a2_in = nc.dram_tensor("a2a2_in", [2, 4 * 128, 256], BF16)
    a2a2_out = nc.dram_tensor("a2a2_out", [2, 4 * 128, 256], BF16)

    with tile.TileContext(nc) as tc, ExitStack() as ctx:
        sb = ctx.enter_context(tc.tile_pool(name="sb", bufs=1))
        # PSUM: 3 x [128,1024] f32 (6 banks) + 2 x [128,512] (2 banks) = 8 banks
        psc = ctx.enter_context(tc.tile_pool(name="psc", bufs=3, space="PSUM"))
        ppv = ctx.enter_context(tc.tile_pool(name="ppv", bufs=2, space="PSUM"))

        xtC = [sb.tile([128, NFT, BQ], BF16, tag=f"xt{c}", name=f"xtC{c}") for c in range(NQB)]
        wq_s = sb.tile([128, NFT * 128], BF16, tag="wq", name="wq_s")
        wk_s = sb.tile([128, NFT * 128], BF16, tag="wk", name="wk_s")
        wv_s = sb.tile([128, NFT * 128], BF16, tag="wv", name="wv_s")
        wo_s = sb.tile([128, NFT, D], BF16, tag="wo", name="wo_s")
        cos_s = sb.tile([128, S], BF16, tag="cos", name="cos_s")
        sin_s = sb.tile([128, S], BF16, tag="sin", name="sin_s")
        lu_s = sb.tile([128, 128], BF16, tag="lu", name="lu_s")
        qTc = [sb.tile([128, BQ], BF16, tag=f"qT{c}", name=f"qTc{c}") for c in range(NQB)]
        # packed key tiles: rows 0-63 = head-0 k-dims, 64-127 = head-1; score
        # matmuls contract K=64 per head as two concurrent row-tiled matmuls
        # (tile_position (0,0) / (64,0)) so both heads' scores cost one pass
        kTc = [sb.tile([128, BQ], BF16, tag=f"kT{c}", name=f"kTc{c}") for c in range(NQB)]
        v_aug = sb.tile([128, NKT, 256], BF16, tag="vaug", name="v_aug")
        aT1 = [sb.tile([128, 256], BF16, tag=f"aT1_{t}", name=f"aT1_{t}") for t in range(NFT)]
        aT2 = [sb.tile([128, 256], BF16, tag=f"aT2_{t}", name=f"aT2_{t}") for t in range(NFT)]

        # startup DMAs: each x chunk is host-packed contiguous in DRAM (2KB+
        # packets), split across queues so the first chunks land ~6us in and
        # later chunks stream in ahead of their consumption (~7us/chunk)
        NH = NFT // 2
        nc.sync.dma_start(xtC[0][:, 0:NH, :], xP[0, :, 0:NH, :])
        nc.sync.dma_start(xtC[1][:, 0:NH, :], xP[1, :, 0:NH, :])
        nc.sync.dma_start(xtC[2][:], xP[2, :, :, :])
        nc.sync.dma_start(xtC[4][:], xP[4, :, :, :])
        nc.sync.dma_start(xtC[6][:], xP[6, :, :, :])
        nc.scalar.dma_start(xtC[0][:, NH:, :], xP[0, :, NH:, :])
        nc.scalar.dma_start(cos_s[:, 0 : S // 2], cosP[:, 0 : S // 2])
        nc.scalar.dma_start(sin_s[:, 0 : S // 2], sinN[:, 0 : S // 2])
        nc.scalar.dma_start(xtC[1][:, NH:, :], xP[1, :, NH:, :])
        nc.scalar.dma_start(xtC[3][:], xP[3, :, :, :])
        nc.scalar.dma_start(cos_s[:, S // 2 : S], cosP[:, S // 2 : S])
        nc.scalar.dma_start(sin_s[:, S // 2 : S], sinN[:, S // 2 : S])
        nc.scalar.dma_start(xtC[5][:], xP[5, :, :, :])
        nc.gpsimd.dma_start(wq_s[:], wq[:, :])
        nc.gpsimd.dma_start(wk_s[:], wk[:, :])
        nc.gpsimd.dma_start(wv_s[:], wv[:, :])
        nc.gpsimd.dma_start(lu_s[:], lu[:, :])
        nc.gpsimd.dma_start(xtC[7][:], xP[7, :, :, :])
        nc.gpsimd.dma_start(wo_s[:], wo[:, :, :])

        # sel: broadcast-matmul stationary — a single ones-row; two col-tiled
        # K=1 matmuls broadcast the head-0/head-1 inverse denominators (packed
        # in one partition-0 row) across output partitions 0-63 / 64-127
        sel = sb.tile([128, 128], BF16, tag="sel", name="sel")
        nc.vector.memset(sel[:], 0.0)
        nc.vector.memset(sel[0:1, 0:64], 1.0)
        wmv = sb.tile([128, BQ], BF16, tag="wmv", name="wmv")
        nc.vector.memset(wmv[:], 1.0)

        # PE warm-up: sustained dummy matmuls while the x/weight DMAs are in
        # flight flip the HAM clock gate to 2.4 GHz before the first real
        # projection
        warm = psc.tile([128, BQ], F32, tag="sc", name="warm")
        for w in range(12):
            nc.tensor.matmul(warm[:], sel[:], wmv[:], start=(w == 0), stop=(w == 11))

        # v_aug: per key tile, two 128-wide stationary blocks (one per head):
        # [64 v dims | ones | 63 zero pad]; 128 columns keep LDWEIGHTS on the
        # fast-weight-load path
        nc.gpsimd.memset(v_aug[:, :, 64:128], 0.0)
        nc.gpsimd.memset(v_aug[:, :, 192:256], 0.0)
        nc.gpsimd.memset(v_aug[:, :, 64:65], 1.0)
        nc.gpsimd.memset(v_aug[:, :, 192:193], 1.0)

        def _piece_qk(nb, is_q):
            acs = slice(BQ * nb, BQ * (nb + 1))
            w_s = wq_s if is_q else wk_s
            p = psc.tile([128, BQ], F32, tag="sc", name="p_qk")
            for t in range(NFT):
                nc.tensor.matmul(
                    p[:],
                    w_s[:, 128 * t : 128 * (t + 1)],
                    xtC[nb][:, t, :],
                    start=(t == 0),
                    stop=(t == NFT - 1),
                )
            a = sb.tile([128, BQ], BF16, tag="ropeA", name="rope_a", bufs=3)
            nc.vector.tensor_copy(a[:], p[:])
            b = sb.tile([128, BQ], BF16, tag="ropeB", name="rope_b", bufs=3)
            nc.vector.stream_shuffle(b[:], a[:], SHUF_HALF)
            t1 = sb.tile([128, BQ], BF16, tag="ropeT", name="rope_t", bufs=3)
            nc.vector.tensor_mul(t1[:], a[:], cos_s[:, acs])
            nc.vector.tensor_mul(b[:], b[:], sin_s[:, acs])
            if is_q:
                nc.vector.tensor_add(qTc[nb][:], t1[:], b[:])
            else:
                nc.vector.tensor_add(kTc[nb][:], t1[:], b[:])

        def _piece_v(st):
            nb, sto = st // 4, st % 4
            pv = psc.tile([128, KT], F32, tag="sc", name="p_v")
            for t in range(NFT):
                nc.tensor.matmul(
                    pv[:],
                    xtC[nb][:, t, KT * sto : KT * (sto + 1)],
                    wv_s[:, 128 * t : 128 * (t + 1)],
                    start=(t == 0),
                    stop=(t == NFT - 1),
                )
            nc.vector.tensor_copy(v_aug[:, st, 0:64], pv[:, 0:64])
            nc.vector.tensor_copy(v_aug[:, st, 128:192], pv[:, 64:128])

        def phase_a_pieces(nb):
            """Projection work for chunk nb as drip-schedulable closures."""
            return (
                [(nb, lambda nb=nb: _piece_qk(nb, True)), (nb, lambda nb=nb: _piece_qk(nb, False))]
                + [(nb, lambda st=st: _piece_v(st)) for st in range(4 * nb, 4 * nb + 4)]
            )

        def phase_a(nb):
            for _, piece in phase_a_pieces(nb):
                piece()

        def phase_b(Q, hooks_prev=(), aqueue=None):
            done_prev = []
            """Attention for query block Q (both heads, all 128x128-mode MMs).
            Returns a closure that emits the deferred normalize+stage for this
            block; the caller runs it once the next block's scores are queued
            so the broadcast matmul never head-of-line-blocks the PE."""
            q0 = BQ * Q
            n_jt = (q0 + BQ) // KT
            n_pair = n_jt // 2
            outT = [
                ppv.tile([128, BQ], F32, tag="pv", name=f"outT{Q}_h{h}") for h in (0, 1)
            ]
            for pr in range(n_pair):
                jts = (2 * pr, 2 * pr + 1)
                nj = 2
                sch = [
                    psc.tile([128, 2 * BQ], F32, tag="sc", name=f"sc_h{h}") for h in (0, 1)
                ]
                for j, jt in enumerate(jts):
                    for h in (0, 1):
                        nc.tensor.matmul(
                            sch[h][:, BQ * j : BQ * (j + 1)],
                            kTc[jt // 4][64 * h : 64 * (h + 1), 128 * (jt % 4) : 128 * (jt % 4 + 1)],
                            qTc[Q][64 * h : 64 * (h + 1), :],
                            start=True,
                            stop=True,
                        )
                for j, jt in enumerate(jts):
                    if KT * jt >= q0:
                        off = KT * jt - q0
                        for h in (0, 1):
                            nc.vector.tensor_add(
                                sch[h][:, BQ * j + off : BQ * j + off + 128],
                                sch[h][:, BQ * j + off : BQ * j + off + 128],
                                lu_s[:],
                            )
                ex = []
                for h in (0, 1):
                    e = sb.tile([128, 2 * BQ], BF16, tag="expT", name=f"expT_h{h}", bufs=6)
                    nc.scalar.activation(
                        e[:, 0 : nj * BQ],
                        sch[h][:, 0 : nj * BQ],
                        mybir.ActivationFunctionType.Exp,
                        scale=0.125,
                    )
                    ex.append(e)
                # deferred work drips in AFTER this pair's exp so its ACT/DVE
                # ops never queue ahead of the exp stream
                due = [h for slot, h in hooks_prev if pr >= slot and h not in done_prev]
                if due:
                    due[0]()
                    done_prev.append(due[0])
                elif aqueue:
                    _, piece = aqueue.pop(0)
                    piece()
                first = pr == 0
                last = pr == n_pair - 1
                for j, jt in enumerate(jts):
                    # columns left of the diagonal block are fully masked but
                    # carry garbage exp values -- PV must skip them
                    trim = max(0, KT * jt - q0)
                    for h in (0, 1):
                        nc.tensor.matmul(
                            outT[h][:, trim:BQ],
                            v_aug[:, jt, 128 * h : 128 * (h + 1)],
                            ex[h][:, BQ * j + trim : BQ * (j + 1)],
                            start=(first and j == 0),
                            stop=(last and j == nj - 1),
                        )
            # evacuate outT to SBUF right away so the PSUM banks free for the
            # next block's PV; the rest of the normalize is deferred
            ov0 = sb.tile([64, BQ], F32, tag="ov0", name="ov0", bufs=3)
            # head-1 values live at partitions 64-127 (same base as their bc
            # rows -- SBUF/SBUF ops need equal input bases)
            ov1 = sb.tile([128, BQ], F32, tag="ov1", name="ov1", bufs=3)
            # den pack: cols 0-511 = den0, 512-1023 = den1, on partition 0
            # (the custom-DVE reciprocal only works at partition base 0)
            dpack = sb.tile([1, 2 * BQ], F32, tag="dpack", name="dpack", bufs=1)
            rbff = sb.tile([1, 2 * BQ], F32, tag="rbff", name="rbff", bufs=1)
            rb1 = sb.tile([1, 2 * BQ], BF16, tag="rb1", name="rb1", bufs=1)

            def evac():
                nc.vector.tensor_copy(ov0[:, :], outT[0][0:64, :])
                nc.vector.tensor_copy(ov1[64:128, :], outT[1][0:64, :])
                nc.vector.tensor_copy(dpack[0:1, 0:BQ], outT[0][64:65, :])
                nc.vector.tensor_copy(dpack[0:1, BQ:], outT[1][64:65, :])

            def recip():
                nc.vector.reciprocal_approx_fast(rbff[0:1, :], dpack[0:1, :])
                nc.vector.tensor_copy(rb1[0:1, :], rbff[0:1, :])

            def finish():
                bcp = psc.tile([128, BQ], F32, tag="sc", name="bcp")
                nc.tensor.matmul(bcp[0:64, :], sel[0:1, 0:64], rb1[0:1, 0:BQ], start=True, stop=True)
                nc.tensor.matmul(bcp[64:128, :], sel[0:1, 0:64], rb1[0:1, BQ:], start=True, stop=True)
                bc = sb.tile([128, BQ], F32, tag="bc", name="bc", bufs=3)
                nc.vector.tensor_copy(bc[:], bcp[:])
                attc = sb.tile([128, BQ], BF16, tag="attc", name="attc", bufs=3)
                nc.vector.tensor_mul(attc[0:64, :], ov0[:, :], bc[0:64, :])
                nc.vector.tensor_mul(attc[64:128, :], ov1[64:128, :], bc[64:128, :])
                tgt, bQ = (a2a1_in, Q) if Q < 4 else (a2a2_in, Q - 4)
                nc.sync.dma_start(tgt[0, 128 * bQ : 128 * (bQ + 1), :], attc[:, 0:256])
                nc.scalar.dma_start(tgt[1, 128 * bQ : 128 * (bQ + 1), :], attc[:, 256:512])

            return evac, recip, finish

        def _cc(a_in, a_out):
            nc.gpsimd.collective_compute(
                "AllToAll",
                mybir.AluOpType.bypass,
                replica_groups=[list(range(N_CORES))],
                ins=[a_in.ap().opt()],
                outs=[a_out.ap().opt()],
            )

        def _load_aT(aTg, a_out, qs):
            for t in range(NFT):
                qs[t % len(qs)].dma_start(
                    aTg[t][:], a_out[t // 4, 128 * (t % 4) : 128 * (t % 4) + 128, :]
                )

        def _wo_piece(g, it, oh, qs):
            aTg = aT1 if g == 0 else aT2
            p = psc.tile([128, 512], F32, tag="sc", name="p_o")
            for t in range(NFT):
                nc.tensor.matmul(
                    p[:],
                    aTg[t][:, 128 * it : 128 * (it + 1)],
                    wo_s[:, t, 512 * oh : 512 * (oh + 1)],
                    start=(t == 0),
                    stop=(t == NFT - 1),
                )
            ot = sb.tile([128, 512], F32, tag="oflush", name="ot", bufs=2)
            nc.vector.tensor_copy(ot[:], p[:])
            r0 = 256 * g + 128 * it
            qs[(2 * it + oh) % len(qs)].dma_start(
                out[r0 : r0 + 128, 512 * oh : 512 * (oh + 1)], ot[:]
            )

        phase_a(0)
        hooks = ()
        aqueue = list(phase_a_pieces(1))
        for Q in range(NQB):
            if Q + 2 < NQB:
                aqueue.extend(phase_a_pieces(Q + 2))
            # anything chunk Q still queued must land before its attention
            while aqueue and aqueue[0][0] <= Q:
                aqueue.pop(0)[1]()
            ev, rc, fin = phase_b(Q, hooks, aqueue)
            hooks = ((0, ev), (1, rc), (3, fin))
            if Q == 4:
                # blocks 0-3 staged (fin(3) dripped during this block): fire
                # collective #1 now so its skew/setup hides under blocks 5-7;
                # aT1 loads go on gpsimd only (a sync/scalar enqueue would
                # head-of-line-block staging DMAs / the exp stream until the
                # collective completes)
                _cc(a2a1_in, a2a1_out)
                _load_aT(aT1, a2a1_out, [nc.gpsimd])
                aqueue.extend(
                    (8, lambda it=it: _wo_piece(0, it, 0, [nc.sync, nc.gpsimd]))
                    for it in (0, 1)
                )
        # two Wo#1 pieces held back: they fill the PE while block 7's
        # normalize chain (DVE) and the collective-#2 rendezvous run
        ev()
        rc()
        _wo_piece(0, 0, 1, [nc.sync, nc.gpsimd])
        fin()
        _wo_piece(0, 1, 1, [nc.sync, nc.gpsimd])
        while aqueue:
            aqueue.pop(0)[1]()

        _cc(a2a2_in, a2a2_out)
        _load_aT(aT2, a2a2_out, [nc.sync, nc.scalar, nc.gpsimd])
        for it in (0, 1):
            for oh in (0, 1):
                _wo_piece(1, it, oh, [nc.sync, nc.scalar, nc.gpsimd])

    nc.compile()
    return nc


def _host_prep(x, Wq, Wk, Wv, Wo):
    bf = ml_dtypes.bfloat16
    # rotary partner permutation: within each head, arrange the 64 dims so a
    # rotation partner is 16 partitions away inside the same 32-block:
    # [e0..e15, o0..o15, e16..e31, o16..o31]
    perm = np.concatenate(
        [
            np.arange(0, 32, 2),
            np.arange(1, 32, 2),
            np.arange(32, 64, 2),
            np.arange(33, 64, 2),
        ]
    )
    pp = np.arange(64)
    pair_i = np.where(pp < 16, pp, np.where(pp < 48, pp - 16, pp - 32))
    sign = np.where((pp % 32) < 16, -1.0, 1.0).astype(np.float32)

    inv_freq = 1.0 / (10000.0 ** (np.arange(0, HD, 2, dtype=np.float32) / HD))
    fr = np.outer(np.arange(S, dtype=np.float32), inv_freq)  # [S, 32]
    cosA = np.cos(fr).T  # [32, S]
    sinA = np.sin(fr).T
    cos64 = cosA[pair_i]
    sin64 = sinA[pair_i] * sign[:, None]
    cosP = np.tile(cos64, (2, 1)).astype(bf)
    sinN = np.tile(sin64, (2, 1)).astype(bf)
    lu = np.tril(np.full((128, 128), -400.0, np.float32), k=-1).astype(bf)

    # xP[c, p, t, s'] = x[512c+s', 128t+p]  (each chunk contiguous in DRAM)
    xP = np.ascontiguousarray(
        np.asarray(x, np.float32).reshape(NQB, BQ, NFT, 128).transpose(0, 3, 2, 1)
    ).astype(bf)
    # woP[p, t, d] = Wo[d, 128t+p]
    woP = np.ascontiguousarray(
        np.asarray(Wo, np.float32).reshape(D, NFT, 128).transpose(2, 1, 0)
    ).astype(bf)

    def pack_w(Wm, rows):
        # w_s[p, 128t+m] = W[rows][128t+p, m]  (transposed shard, feature-major)
        wT = np.asarray(Wm, np.float32)[rows].T  # [D, 128]
        return np.ascontiguousarray(
            wT.reshape(NFT, 128, 128).transpose(1, 0, 2).reshape(128, NFT * 128)
        ).astype(bf)

    in_maps = []
    for c in range(N_CORES):
        rows = np.concatenate([128 * c + 64 * h + perm for h in range(2)])
        vrows = np.arange(128 * c, 128 * (c + 1))
        in_maps.append(
            {
                "xP": xP,
                "wq": pack_w(Wq, rows),
                "wk": pack_w(Wk, rows),
                "wv": pack_w(Wv, vrows),
                "wo": woP,
                "cosP": cosP,
                "sinN": sinN,
                "lu": lu,
            }
        )
    return in_maps


_NC_CACHE = None


def _assemble(results):
    # core c owns queries 512*(c%4) + 256*(c//4) + [0,256) of each 2048-query
    # half (from AllToAll #1 / #2 respectively)
    full = np.empty((S, D), np.float32)
    for c in range(N_CORES):
        q0 = 512 * (c % 4) + 256 * (c // 4)
        full[q0 : q0 + 256] = results[c]["out"][0:256]
        full[2048 + q0 : 2048 + q0 + 256] = results[c]["out"][256:512]
    return full.reshape(1, S, D)


def kernel(x, Wq, Wk, Wv, Wo):
    global _NC_CACHE
    if _NC_CACHE is None:
        _NC_CACHE = _build()
    nc = _NC_CACHE
    in_maps = _host_prep(
        np.asarray(x, np.float32),
        np.asarray(Wq, np.float32),
        np.asarray(Wk, np.float32),
        np.asarray(Wv, np.float32),
        np.asarray(Wo, np.float32),
    )
    res = run_bass_kernel_spmd(nc, in_maps, core_ids=list(range(N_CORES)))
    return _assemble(res.results)


# revision 32
# speedup vs baseline: 1.0791x; 1.0791x over previous
"""nn_MultiHeadAttention TRN2 kernel: 8-core tensor-parallel (2 heads/core).

Self-contained: builds and compiles the Bass/Tile SPMD program on first call,
shards the full inputs per-core on the host, runs via run_bass_kernel_spmd,
and concatenates the per-core sequence-block outputs into the full output.

Algorithm (per core, 2 heads of 16, head_dim 64, S=4096, D=1024):
  - feature-major layout: xP [128, 8, S] (partition, feature-tile, seq) so one
    1MB DMA delivers a full 512-query chunk across all 8 feature tiles;
    projections for chunk 0 start ~4us in.  Weights are host-packed so each
    loads in a single descriptor.
  - q/k projected with RoPE-permuted, transposed weight shards; the rotary
    partner permutation makes the rotation a 16-lane half-swap within every
    32-partition block (one DVE stream_shuffle), signs folded into sin.
  - chunked pipeline: per 512-query chunk, project q/k (PE), evacuate+RoPE
    (DVE), project v into an augmented seq-major tile with a ones column
    (softmax denominator); attention for query block Q starts as soon as its
    chunks are ready.
  - flash attention on transposed score tiles scoresT[k,q]: scores for a
    jt-pair land in a [128,1024] PSUM tile (3-slot rotation), causal mask
    added as a -400 triangle on diagonal tiles (DVE), exp on ScalarE over
    [128,1024] per head-pair, PV accumulates outT[128,512] per head (row 64
    = softmax denominator, rows 65-127 padding from the 128-wide stationary).
  - normalize, staged over the next block so it never blocks the PE or DVE:
    outT evacuates to SBUF immediately (PSUM recycles), two 1-row DVE
    reciprocals produce the inverse denominators, one bf16 128x128
    select-matmul broadcasts them across partitions, two DVE multiplies;
    per-block staging DMA feeds the AllToAll input.
  - startup DMAs/memsets spread across the SP/ACT/DVE/GpSimd queues (the
    ~0.6us per-descriptor enqueue is serial per queue).
  - AllToAll re-shards from head-split to sequence-split; final projection
    against full Wo.T; each core emits out[512, 1024] f32.
"""

from contextlib import ExitStack

import numpy as np
import ml_dtypes

import concourse.tile as tile
from concourse import bacc, mybir
from concourse.bass_utils import run_bass_kernel_spmd

F32 = mybir.dt.float32
BF16 = mybir.dt.bfloat16

S = 4096
D = 1024
HD = 64
N_CORES = 8
KT = 128
BQ = 512
NFT = D // 128          # 8 feature tiles
NKT = S // KT           # 32 key tiles
NQB = S // BQ           # 8 query blocks
CHUNK = S // N_CORES    # 512

SHUF_HALF = list(range(16, 32)) + list(range(16))


def _build():
    nc = bacc.Bacc("TRN2", target_bir_lowering=False, debug=False, num_devices=N_CORES)

    xP = nc.dram_tensor("xP", [NQB, 128, NFT, BQ], BF16, kind="ExternalInput")
    wq = nc.dram_tensor("wq", [128, NFT * 128], BF16, kind="ExternalInput")
    wk = nc.dram_tensor("wk", [128, NFT * 128], BF16, kind="ExternalInput")
    wv = nc.dram_tensor("wv", [128, NFT * 128], BF16, kind="ExternalInput")
    wo = nc.dram_tensor("wo", [128, NFT, D], BF16, kind="ExternalInput")
    cosP = nc.dram_tensor("cosP", [128, S], BF16, kind="ExternalInput")
    sinN = nc.dram_tensor("sinN", [128, S], BF16, kind="ExternalInput")
    lu = nc.dram_tensor("lu", [128, 128], BF16, kind="ExternalInput")
    out = nc.dram_tensor("out", [CHUNK, D], F32, kind="ExternalOutput")

    # split AllToAll: #1 covers query blocks 0-3 (fires mid-kernel, hides the
    # inter-core skew + CC setup behind attention of blocks 4-7), #2 covers
    # blocks 4-7 in the tail.  Layout [2, 512, 256]: chunk d = (query-half
    # d//4, block d%4) -> core d owns queries 512*(d%4) + 256*(d//4) + [0,256)
    # of each 2048-query group.
    a2a1_in = nc.dram_tensor("a2a1_in", [2, 4 * 128, 256], BF16)
    a2a1_out = nc.dram_tensor("a2a1_out", [2, 4 * 128, 256], BF16)
    a2a2_in = nc.dram_tensor("a2a2_in", [2, 4 * 128, 256], BF16)
    a2a2_out = nc.dram_tensor("a2a2_out", [2, 4 * 128, 256], BF16)

    with tile.TileContext(nc) as tc, ExitStack() as ctx:
        sb = ctx.enter_context(tc.tile_pool(name="sb", bufs=1))
        # PSUM: 3 x [128,1024] f32 (6 banks) + 2 x [128,512] (2 banks) = 8 banks
        psc = ctx.enter_context(tc.tile_pool(name="psc", bufs=3, space="PSUM"))
        ppv = ctx.enter_context(tc.tile_pool(name="ppv", bufs=2, space="PSUM"))

        xtC = [sb.tile([128, NFT, BQ], BF16, tag=f"xt{c}", name=f"xtC{c}") for c in range(NQB)]
        wq_s = sb.tile([128, NFT * 128], BF16, tag="wq", name="wq_s")
        wk_s = sb.tile([128, NFT * 128], BF16, tag="wk", name="wk_s")
        wv_s = sb.tile([128, NFT * 128], BF16, tag="wv", name="wv_s")
        wo_s = sb.tile([128, NFT, D], BF16, tag="wo", name="wo_s")
        cos_s = sb.tile([128, S], BF16, tag="cos", name="cos_s")
        sin_s = sb.tile([128, S], BF16, tag="sin", name="sin_s")
        lu_s = sb.tile([128, 128], BF16, tag="lu", name="lu_s")
        qTc = [sb.tile([128, BQ], BF16, tag=f"qT{c}", name=f"qTc{c}") for c in range(NQB)]
        # packed key tiles: rows 0-63 = head-0 k-dims, 64-127 = head-1; score
        # matmuls contract K=64 per head as two concurrent row-tiled matmuls
        # (tile_position (0,0) / (64,0)) so both heads' scores cost one pass
        kTc = [sb.tile([128, BQ], BF16, tag=f"kT{c}", name=f"kTc{c}") for c in range(NQB)]
        v_aug = sb.tile([128, NKT, 256], BF16, tag="vaug", name="v_aug")
        aT1 = [sb.tile([128, 256], BF16, tag=f"aT1_{t}", name=f"aT1_{t}") for t in range(NFT)]
        aT2 = [sb.tile([128, 256], BF16, tag=f"aT2_{t}", name=f"aT2_{t}") for t in range(NFT)]

        # startup DMAs: each x chunk is host-packed contiguous in DRAM (2KB+
        # packets), split across queues so the first chunks land ~6us in and
        # later chunks stream in ahead of their consumption (~7us/chunk)
        NH = NFT // 2
        nc.sync.dma_start(xtC[0][:, 0:NH, :], xP[0, :, 0:NH, :])
        nc.sync.dma_start(xtC[1][:, 0:NH, :], xP[1, :, 0:NH, :])
        nc.sync.dma_start(xtC[2][:], xP[2, :, :, :])
        nc.sync.dma_start(xtC[4][:], xP[4, :, :, :])
        nc.sync.dma_start(xtC[6][:], xP[6, :, :, :])
        nc.scalar.dma_start(xtC[0][:, NH:, :], xP[0, :, NH:, :])
        nc.scalar.dma_start(cos_s[:, 0 : S // 2], cosP[:, 0 : S // 2])
        nc.scalar.dma_start(sin_s[:, 0 : S // 2], sinN[:, 0 : S // 2])
        nc.scalar.dma_start(xtC[1][:, NH:, :], xP[1, :, NH:, :])
        nc.scalar.dma_start(xtC[3][:], xP[3, :, :, :])
        nc.scalar.dma_start(cos_s[:, S // 2 : S], cosP[:, S // 2 : S])
        nc.scalar.dma_start(sin_s[:, S // 2 : S], sinN[:, S // 2 : S])
        nc.scalar.dma_start(xtC[5][:], xP[5, :, :, :])
        nc.gpsimd.dma_start(wq_s[:], wq[:, :])
        nc.gpsimd.dma_start(wk_s[:], wk[:, :])
        nc.gpsimd.dma_start(wv_s[:], wv[:, :])
        nc.gpsimd.dma_start(lu_s[:], lu[:, :])
        nc.gpsimd.dma_start(xtC[7][:], xP[7, :, :, :])
        nc.gpsimd.dma_start(wo_s[:], wo[:, :, :])

        # sel: broadcast-matmul stationary — a single ones-row; two col-tiled
        # K=1 matmuls broadcast the head-0/head-1 inverse denominators (packed
        # in one partition-0 row) across output partitions 0-63 / 64-127
        sel = sb.tile([128, 128], BF16, tag="sel", name="sel")
        nc.vector.memset(sel[:], 0.0)
        nc.vector.memset(sel[0:1, 0:64], 1.0)
        wmv = sb.tile([128, BQ], BF16, tag="wmv", name="wmv")
        nc.vector.memset(wmv[:], 1.0)

        # PE warm-up: sustained dummy matmuls while the x/weight DMAs are in
        # flight flip the HAM clock gate to 2.4 GHz before the first real
        # projection
        warm = psc.tile([128, BQ], F32, tag="sc", name="warm")
        for w in range(12):
            nc.tensor.matmul(warm[:], sel[:], wmv[:], start=(w == 0), stop=(w == 11))

        # v_aug: per key tile, two 128-wide stationary blocks (one per head):
        # [64 v dims | ones | 63 zero pad]; 128 columns keep LDWEIGHTS on the
        # fast-weight-load path
        nc.gpsimd.memset(v_aug[:, :, 64:128], 0.0)
        nc.gpsimd.memset(v_aug[:, :, 192:256], 0.0)
        nc.gpsimd.memset(v_aug[:, :, 64:65], 1.0)
        nc.gpsimd.memset(v_aug[:, :, 192:193], 1.0)

        def _piece_qk(nb, is_q):
            acs = slice(BQ * nb, BQ * (nb + 1))
            w_s = wq_s if is_q else wk_s
            p = psc.tile([128, BQ], F32, tag="sc", name="p_qk")
            for t in range(NFT):
                nc.tensor.matmul(
                    p[:],
                    w_s[:, 128 * t : 128 * (t + 1)],
                    xtC[nb][:, t, :],
                    start=(t == 0),
                    stop=(t == NFT - 1),
                )
            a = sb.tile([128, BQ], BF16, tag="ropeA", name="rope_a", bufs=3)
            nc.vector.tensor_copy(a[:], p[:])
            b = sb.tile([128, BQ], BF16, tag="ropeB", name="rope_b", bufs=3)
            nc.vector.stream_shuffle(b[:], a[:], SHUF_HALF)
            t1 = sb.tile([128, BQ], BF16, tag="ropeT", name="rope_t", bufs=3)
            nc.vector.tensor_mul(t1[:], a[:], cos_s[:, acs])
            nc.vector.tensor_mul(b[:], b[:], sin_s[:, acs])
            if is_q:
                nc.vector.tensor_add(qTc[nb][:], t1[:], b[:])
            else:
                nc.vector.tensor_add(kTc[nb][:], t1[:], b[:])

        def _piece_v(st):
            nb, sto = st // 4, st % 4
            pv = psc.tile([128, KT], F32, tag="sc", name="p_v")
            for t in range(NFT):
                nc.tensor.matmul(
                    pv[:],
                    xtC[nb][:, t, KT * sto : KT * (sto + 1)],
                    wv_s[:, 128 * t : 128 * (t + 1)],
                    start=(t == 0),
                    stop=(t == NFT - 1),
                )
            nc.vector.tensor_copy(v_aug[:, st, 0:64], pv[:, 0:64])
            nc.vector.tensor_copy(v_aug[:, st, 128:192], pv[:, 64:128])

        def phase_a_pieces(nb):
            """Projection work for chunk nb as drip-schedulable closures."""
            return (
                [(nb, lambda nb=nb: _piece_qk(nb, True)), (nb, lambda nb=nb: _piece_qk(nb, False))]
                + [(nb, lambda st=st: _piece_v(st)) for st in range(4 * nb, 4 * nb + 4)]
            )

        def phase_a(nb):
            for _, piece in phase_a_pieces(nb):
                piece()

        def phase_b(Q, hooks_prev=(), aqueue=None):
            done_prev = []
            """Attention for query block Q (both heads, all 128x128-mode MMs).
            Returns a closure that emits the deferred normalize+stage for this
            block; the caller runs it once the next block's scores are queued
            so the broadcast matmul never head-of-line-blocks the PE."""
            q0 = BQ * Q
            n_jt = (q0 + BQ) // KT
            n_pair = n_jt // 2
            outT = [
                ppv.tile([128, BQ], F32, tag="pv", name=f"outT{Q}_h{h}") for h in (0, 1)
            ]
            prev_pv = None
            for pr in range(n_pair):
                jts = (2 * pr, 2 * pr + 1)
                nj = 2
                sch = [
                    psc.tile([128, 2 * BQ], F32, tag="sc", name=f"sc_h{h}") for h in (0, 1)
                ]
                for j, jt in enumerate(jts):
                    for h in (0, 1):
                        nc.tensor.matmul(
                            sch[h][:, BQ * j : BQ * (j + 1)],
                            kTc[jt // 4][64 * h : 64 * (h + 1), 128 * (jt % 4) : 128 * (jt % 4 + 1)],
                            qTc[Q][64 * h : 64 * (h + 1), :],
                            start=True,
                            stop=True,
                        )
                for j, jt in enumerate(jts):
                    if KT * jt >= q0:
                        off = KT * jt - q0
                        for h in (0, 1):
                            nc.vector.tensor_add(
                                sch[h][:, BQ * j + off : BQ * j + off + 128],
                                sch[h][:, BQ * j + off : BQ * j + off + 128],
                                lu_s[:],
                            )
                ex = []
                for h in (0, 1):
                    e = sb.tile([128, 2 * BQ], BF16, tag="expT", name=f"expT_h{h}", bufs=6)
                    nc.scalar.activation(
                        e[:, 0 : nj * BQ],
                        sch[h][:, 0 : nj * BQ],
                        mybir.ActivationFunctionType.Exp,
                        scale=0.125,
                    )
                    ex.append(e)
                # deferred work drips in AFTER this pair's exp so its ACT/DVE
                # ops never queue ahead of the exp stream
                due = [h for slot, h in hooks_prev if pr >= slot and h not in done_prev]
                if due:
                    due[0]()
                    done_prev.append(due[0])
                elif aqueue:
                    _, piece = aqueue.pop(0)
                    piece()
                # software-pipelined PV: emit the PREVIOUS pair's PV here so
                # this pair's scores sit ahead of it in the in-order tensor
                # queue -- the PE runs a pair ahead of the exp stream instead
                # of head-of-line-blocking on ACT
                first = pr == 0
                last = pr == n_pair - 1

                def pv_emit(jts=jts, ex=ex, first=first, last=last):
                    for j, jt in enumerate(jts):
                        # columns left of the diagonal block are fully masked
                        # but carry garbage exp values -- PV must skip them
                        trim = max(0, KT * jt - q0)
                        for h in (0, 1):
                            nc.tensor.matmul(
                                outT[h][:, trim:BQ],
                                v_aug[:, jt, 128 * h : 128 * (h + 1)],
                                ex[h][:, BQ * j + trim : BQ * (j + 1)],
                                start=(first and j == 0),
                                stop=(last and j == nj - 1),
                            )

                if prev_pv is not None:
                    prev_pv()
                prev_pv = pv_emit
            prev_pv()
            # evacuate outT to SBUF right away so the PSUM banks free for the
            # next block's PV; the rest of the normalize is deferred
            ov0 = sb.tile([64, BQ], F32, tag="ov0", name="ov0", bufs=3)
            # head-1 values live at partitions 64-127 (same base as their bc
            # rows -- SBUF/SBUF ops need equal input bases)
            ov1 = sb.tile([128, BQ], F32, tag="ov1", name="ov1", bufs=3)
            # den pack: cols 0-511 = den0, 512-1023 = den1, on partition 0
            # (the custom-DVE reciprocal only works at partition base 0)
            dpack = sb.tile([1, 2 * BQ], F32, tag="dpack", name="dpack", bufs=1)
            rbff = sb.tile([1, 2 * BQ], F32, tag="rbff", name="rbff", bufs=1)
            rb1 = sb.tile([1, 2 * BQ], BF16, tag="rb1", name="rb1", bufs=1)

            def evac():
                nc.vector.tensor_copy(ov0[:, :], outT[0][0:64, :])
                nc.vector.tensor_copy(ov1[64:128, :], outT[1][0:64, :])
                nc.vector.tensor_copy(dpack[0:1, 0:BQ], outT[0][64:65, :])
                nc.vector.tensor_copy(dpack[0:1, BQ:], outT[1][64:65, :])

            def recip():
                nc.vector.reciprocal_approx_fast(rbff[0:1, :], dpack[0:1, :])
                nc.vector.tensor_copy(rb1[0:1, :], rbff[0:1, :])

            def finish():
                bcp = psc.tile([128, BQ], F32, tag="sc", name="bcp")
                nc.tensor.matmul(bcp[0:64, :], sel[0:1, 0:64], rb1[0:1, 0:BQ], start=True, stop=True)
                nc.tensor.matmul(bcp[64:128, :], sel[0:1, 0:64], rb1[0:1, BQ:], start=True, stop=True)
                bc = sb.tile([128, BQ], F32, tag="bc", name="bc", bufs=3)
                nc.vector.tensor_copy(bc[:], bcp[:])
                attc = sb.tile([128, BQ], BF16, tag="attc", name="attc", bufs=3)
                nc.vector.tensor_mul(attc[0:64, :], ov0[:, :], bc[0:64, :])
                nc.vector.tensor_mul(attc[64:128, :], ov1[64:128, :], bc[64:128, :])
                tgt, bQ = (a2a1_in, Q) if Q < 4 else (a2a2_in, Q - 4)
                nc.sync.dma_start(tgt[0, 128 * bQ : 128 * (bQ + 1), :], attc[:, 0:256])
                nc.scalar.dma_start(tgt[1, 128 * bQ : 128 * (bQ + 1), :], attc[:, 256:512])

            return evac, recip, finish

        def _cc(a_in, a_out):
            nc.gpsimd.collective_compute(
                "AllToAll",
                mybir.AluOpType.bypass,
                replica_groups=[list(range(N_CORES))],
                ins=[a_in.ap().opt()],
                outs=[a_out.ap().opt()],
            )

        def _load_aT(aTg, a_out, qs):
            for t in range(NFT):
                qs[t % len(qs)].dma_start(
                    aTg[t][:], a_out[t // 4, 128 * (t % 4) : 128 * (t % 4) + 128, :]
                )

        def _wo_piece(g, it, oh, qs):
            aTg = aT1 if g == 0 else aT2
            p = psc.tile([128, 512], F32, tag="sc", name="p_o")
            for t in range(NFT):
                nc.tensor.matmul(
                    p[:],
                    aTg[t][:, 128 * it : 128 * (it + 1)],
                    wo_s[:, t, 512 * oh : 512 * (oh + 1)],
                    start=(t == 0),
                    stop=(t == NFT - 1),
                )
            ot = sb.tile([128, 512], F32, tag="oflush", name="ot", bufs=2)
            nc.vector.tensor_copy(ot[:], p[:])
            r0 = 256 * g + 128 * it
            qs[(2 * it + oh) % len(qs)].dma_start(
                out[r0 : r0 + 128, 512 * oh : 512 * (oh + 1)], ot[:]
            )

        phase_a(0)
        hooks = ()
        aqueue = list(phase_a_pieces(1))
        for Q in range(NQB):
            if Q + 2 < NQB:
                aqueue.extend(phase_a_pieces(Q + 2))
            # anything chunk Q still queued must land before its attention
            while aqueue and aqueue[0][0] <= Q:
                aqueue.pop(0)[1]()
            ev, rc, fin = phase_b(Q, hooks, aqueue)
            hooks = ((0, ev), (1, rc), (3, fin))
            if Q == 4:
                # blocks 0-3 staged (fin(3) dripped during this block): fire
                # collective #1 now so its skew/setup hides under blocks 5-7;
                # aT1 loads go on gpsimd only (a sync/scalar enqueue would
                # head-of-line-block staging DMAs / the exp stream until the
                # collective completes)
                _cc(a2a1_in, a2a1_out)
                _load_aT(aT1, a2a1_out, [nc.gpsimd])
                aqueue.extend(
                    (8, lambda it=it: _wo_piece(0, it, 0, [nc.sync, nc.gpsimd]))
                    for it in (0, 1)
                )
        # two Wo#1 pieces held back: they fill the PE while block 7's
        # normalize chain (DVE) and the collective-#2 rendezvous run
        ev()
        rc()
        _wo_piece(0, 0, 1, [nc.sync, nc.gpsimd])
        fin()
        _wo_piece(0, 1, 1, [nc.sync, nc.gpsimd])
        while aqueue:
            aqueue.pop(0)[1]()

        _cc(a2a2_in, a2a2_out)
        _load_aT(aT2, a2a2_out, [nc.sync, nc.scalar, nc.gpsimd])
        for it in (0, 1):
            for oh in (0, 1):
                _wo_piece(1, it, oh, [nc.sync, nc.scalar, nc.gpsimd])

    nc.compile()
    return nc


def _host_prep(x, Wq, Wk, Wv, Wo):
    bf = ml_dtypes.bfloat16
    # rotary partner permutation: within each head, arrange the 64 dims so a
    # rotation partner is 16 partitions away inside the same 32-block:
    # [e0..e15, o0..o15, e16..e31, o16..o31]
    perm = np.concatenate(
        [
            np.arange(0, 32, 2),
            np.arange(1, 32, 2),
            np.arange(32, 64, 2),
            np.arange(33, 64, 2),
        ]
    )
    pp = np.arange(64)
    pair_i = np.where(pp < 16, pp, np.where(pp < 48, pp - 16, pp - 32))
    sign = np.where((pp % 32) < 16, -1.0, 1.0).astype(np.float32)

    inv_freq = 1.0 / (10000.0 ** (np.arange(0, HD, 2, dtype=np.float32) / HD))
    fr = np.outer(np.arange(S, dtype=np.float32), inv_freq)  # [S, 32]
    cosA = np.cos(fr).T  # [32, S]
    sinA = np.sin(fr).T
    cos64 = cosA[pair_i]
    sin64 = sinA[pair_i] * sign[:, None]
    cosP = np.tile(cos64, (2, 1)).astype(bf)
    sinN = np.tile(sin64, (2, 1)).astype(bf)
    lu = np.tril(np.full((128, 128), -400.0, np.float32), k=-1).astype(bf)

    # xP[c, p, t, s'] = x[512c+s', 128t+p]  (each chunk contiguous in DRAM)
    xP = np.ascontiguousarray(
        np.asarray(x, np.float32).reshape(NQB, BQ, NFT, 128).transpose(0, 3, 2, 1)
    ).astype(bf)
    # woP[p, t, d] = Wo[d, 128t+p]
    woP = np.ascontiguousarray(
        np.asarray(Wo, np.float32).reshape(D, NFT, 128).transpose(2, 1, 0)
    ).astype(bf)

    def pack_w(Wm, rows):
        # w_s[p, 128t+m] = W[rows][128t+p, m]  (transposed shard, feature-major)
        wT = np.asarray(Wm, np.float32)[rows].T  # [D, 128]
        return np.ascontiguousarray(
            wT.reshape(NFT, 128, 128).transpose(1, 0, 2).reshape(128, NFT * 128)
        ).astype(bf)

    in_maps = []
    for c in range(N_CORES):
        rows = np.concatenate([128 * c + 64 * h + perm for h in range(2)])
        vrows = np.arange(128 * c, 128 * (c + 1))
        in_maps.append(
            {
                "xP": xP,
                "wq": pack_w(Wq, rows),
                "wk": pack_w(Wk, rows),
                "wv": pack_w(Wv, vrows),
                "wo": woP,
                "cosP": cosP,
                "sinN": sinN,
                "lu": lu,
            }
        )
    return in_maps


_NC_CACHE = None


def _assemble(results):
    # core c owns queries 512*(c%4) + 256*(c//4) + [0,256) of each 2048-query
    # half (from AllToAll #1 / #2 respectively)
    full = np.empty((S, D), np.float32)
    for c in range(N_CORES):
        q0 = 512 * (c % 4) + 256 * (c // 4)
        full[q0 : q0 + 256] = results[c]["out"][0:256]
        full[2048 + q0 : 2048 + q0 + 256] = results[c]["out"][256:512]
    return full.reshape(1, S, D)


def kernel(x, Wq, Wk, Wv, Wo):
    global _NC_CACHE
    if _NC_CACHE is None:
        _NC_CACHE = _build()
    nc = _NC_CACHE
    in_maps = _host_prep(
        np.asarray(x, np.float32),
        np.asarray(Wq, np.float32),
        np.asarray(Wk, np.float32),
        np.asarray(Wv, np.float32),
        np.asarray(Wo, np.float32),
    )
    res = run_bass_kernel_spmd(nc, in_maps, core_ids=list(range(N_CORES)))
    return _assemble(res.results)
